# revision 4
# baseline (speedup 1.0000x reference)
"""Causal self-attention (B=2, T=2048, C=1024, 16 heads x 64) on 8 TRN2 NeuronCores.

Tensor-parallel over heads: each core owns 2 heads. Per core:
  - QKV projection computed in transposed layout (features on partitions)
    from a host-pretransposed x^T, so the contraction dim C sits on
    partitions for every matmul.
  - Scores are computed as S^T (keys on partitions, queries on free dim),
    softmax denominator comes for free from a ones-column appended to V,
    masking folds into the exp activation's per-partition bias, and the
    causal triangle is a single 128x128 multiply per diagonal block.
  - Output projection is split-K over the two heads; partial outputs are
    summed on the host (the all-reduce of row-parallel tensor parallelism).
"""

import os
import sys
import types

import numpy as np

for _p in ("/opt/trn_rl_repo",):
    if os.path.isdir(_p) and _p not in sys.path:
        sys.path.append(_p)

import concourse.bass as bass  # noqa: E402
import concourse.tile as tile  # noqa: E402
from concourse import bacc, mybir  # noqa: E402
from concourse.bass_utils import run_bass_kernel_spmd  # noqa: E402
from concourse.masks import make_identity, make_upper_triangular  # noqa: E402

B, T, C = 2, 2048, 1024
NH, HS = 16, 64
NCORES = 8
HPC = NH // NCORES          # heads per core
F = HPC * HS                # per-core qkv feature width (128)
NTOK = B * T                # 4096 flattened tokens
SEG = 512                   # token segment width for the qkv projection
NSEG = NTOK // SEG
NKC = T // 128              # key chunks per batch
QTW = 1024                  # query tile width in the attention loop
NQT = T // QTW
MASK_NEG = -30000.0

F32 = mybir.dt.float32

# "float32r" runs the PE at 4x the float32 rate (moving dim >= 256) with a
# reduced-precision multiply; "float32" is exact. Selected via env for A/B.
DT_MM_NAME = os.environ.get("KERNEL_DT_MM", "float32r")

_prog_cache = {}


def _build(dt_mm_name):
    dt_mm = getattr(mybir.dt, dt_mm_name)

    def mmcast(ap):
        return ap if dt_mm is F32 else ap.bitcast(dt_mm)

    nc = bacc.Bacc("TRN2", target_bir_lowering=False, debug=False)

    xT = nc.declare_dram_parameter("xT", [C, NTOK], F32, isOutput=False)
    wq = nc.declare_dram_parameter("wq", [C, F], F32, isOutput=False)
    wk = nc.declare_dram_parameter("wk", [C, F], F32, isOutput=False)
    wv = nc.declare_dram_parameter("wv", [C, F], F32, isOutput=False)
    bq = nc.declare_dram_parameter("bq", [F, 1], F32, isOutput=False)
    bk = nc.declare_dram_parameter("bk", [F, 1], F32, isOutput=False)
    bv = nc.declare_dram_parameter("bv", [F, 1], F32, isOutput=False)
    wp = nc.declare_dram_parameter("wp", [F, C], F32, isOutput=False)
    mbias = nc.declare_dram_parameter("mbias", [128, B * NKC], F32, isOutput=False)
    out = nc.declare_dram_parameter("out", [NTOK, C], F32, isOutput=True)

    with tile.TileContext(nc) as tc:
        persist = tc.alloc_tile_pool(name="persist", bufs=1)
        xt_pool = tc.alloc_tile_pool(name="xt", bufs=4)
        pt_pool = tc.alloc_tile_pool(name="pt", bufs=3)
        dn_pool = tc.alloc_tile_pool(name="dn", bufs=2)
        ob_pool = tc.alloc_tile_pool(name="ob", bufs=3)

        # --- constants / weights ---
        wqT = persist.tile([128, C // 128, F], F32, tag="wqT")
        wkT = persist.tile([128, C // 128, F], F32, tag="wkT")
        wvT = persist.tile([128, C // 128, F], F32, tag="wvT")
        nc.sync.dma_start(out=wqT, in_=wq[:].rearrange("(k p) f -> p k f", p=128))
        nc.sync.dma_start(out=wkT, in_=wk[:].rearrange("(k p) f -> p k f", p=128))
        nc.sync.dma_start(out=wvT, in_=wv[:].rearrange("(k p) f -> p k f", p=128))
        bqc = persist.tile([F, 1], F32, tag="bqc")
        bkc = persist.tile([F, 1], F32, tag="bkc")
        bvc = persist.tile([F, 1], F32, tag="bvc")
        nc.sync.dma_start(out=bqc, in_=bq[:])
        nc.sync.dma_start(out=bkc, in_=bk[:])
        nc.sync.dma_start(out=bvc, in_=bv[:])
        wp0 = persist.tile([HS, C], F32, tag="wp0")
        wp1 = persist.tile([HS, C], F32, tag="wp1")
        nc.sync.dma_start(out=wp0, in_=wp[0:HS, :])
        nc.sync.dma_start(out=wp1, in_=wp[HS:F, :])
        mb = persist.tile([128, B * NKC], F32, tag="mb")
        nc.sync.dma_start(out=mb, in_=mbias[:])

        ident = persist.tile([128, 128], F32, tag="ident")
        make_identity(nc, ident)
        tri01 = persist.tile([128, 128], F32, tag="tri01")
        make_upper_triangular(nc, tri01, val=1.0, diag=True)

        QT = persist.tile([F, NTOK], F32, tag="QT")
        KT = persist.tile([F, NTOK], F32, tag="KT")
        VT = persist.tile([F, NTOK], F32, tag="VT")

        # --- phase A: q/k/v projections in transposed layout ---
        with tc.tile_pool(name="psA", bufs=2, space="PSUM") as psA:
            for s in range(NSEG):
                sl = slice(s * SEG, (s + 1) * SEG)
                xts = []
                for k in range(C // 128):
                    xt = xt_pool.tile([128, SEG], F32, tag="xt")
                    nc.sync.dma_start(out=xt, in_=xT[k * 128:(k + 1) * 128, sl])
                    xts.append(xt)
                psq = psA.tile([F, SEG], F32, tag="psq")
                psk = psA.tile([F, SEG], F32, tag="psk")
                psv = psA.tile([F, SEG], F32, tag="psv")
                for k in range(C // 128):
                    f = dict(start=(k == 0), stop=(k == C // 128 - 1))
                    nc.tensor.matmul(psq, mmcast(wqT[:, k, :]), mmcast(xts[k][:]), **f)
                    nc.tensor.matmul(psk, mmcast(wkT[:, k, :]), mmcast(xts[k][:]), **f)
                    nc.tensor.matmul(psv, mmcast(wvT[:, k, :]), mmcast(xts[k][:]), **f)
                nc.vector.tensor_scalar_add(QT[:, sl], psq, bqc)
                nc.vector.tensor_scalar_add(KT[:, sl], psk, bkc)
                nc.vector.tensor_scalar_add(VT[:, sl], psv, bvc)

        # --- phase B: V^T -> V' (keys on partitions, ones column for denom) ---
        Vp = {}
        for b in range(B):
            for h in range(HPC):
                Vp[(b, h)] = persist.tile([128, NKC, HS + 1], F32, tag=f"vp{b}{h}", name=f"vp{b}{h}")
                nc.gpsimd.memset(Vp[(b, h)][:, :, HS:HS + 1], 1.0)
        with tc.tile_pool(name="psB", bufs=2, space="PSUM") as psB:
            for b in range(B):
                for h in range(HPC):
                    hp = slice(h * HS, (h + 1) * HS)
                    for k in range(NKC):
                        src = VT[hp, b * T + k * 128: b * T + (k + 1) * 128]
                        tr = psB.tile([128, HS], F32, tag="tr")
                        nc.tensor.transpose(tr, src, ident[hp, hp])
                        nc.vector.tensor_copy(Vp[(b, h)][:, k, 0:HS], tr)

        # --- phases C+D: attention + output projection ---
        yTs = {}
        for b in range(B):
            for h in range(HPC):
                yTs[(b, h)] = persist.tile([HS, T], F32, tag=f"yt{b}{h}", name=f"yt{b}{h}")

        with (
            tc.tile_pool(name="psS", bufs=2, space="PSUM") as psS,
            tc.tile_pool(name="acc", bufs=2, space="PSUM") as acc,
        ):
            for b in range(B):
                for h in range(HPC):
                    hp = slice(h * HS, (h + 1) * HS)
                    for qt in range(NQT):
                        qlo = qt * QTW
                        kmax = (qlo + QTW) // 128
                        kpc1 = min(kmax, qlo // 128 + 4)  # last k touching piece 1
                        y = acc.tile([HS + 1, QTW], F32, tag="acc")
                        for k in range(kmax):
                            c0 = max(0, k * 128 - qlo)
                            pieces = (
                                [(c0, SEG), (SEG, QTW)] if c0 < SEG else [(c0, QTW)]
                            )
                            st = psS.tile([128, QTW], F32, tag="st")
                            for (a, e) in pieces:
                                nc.tensor.matmul(
                                    st[:, a:e],
                                    mmcast(KT[hp, b * T + k * 128: b * T + (k + 1) * 128]),
                                    mmcast(QT[hp, b * T + qlo + a: b * T + qlo + e]),
                                    start=True, stop=True,
                                )
                            pt = pt_pool.tile([128, QTW], F32, tag="pt")
                            nc.scalar.activation(
                                pt[:, c0:QTW], st[:, c0:QTW],
                                func=mybir.ActivationFunctionType.Exp,
                                bias=mb[:, b * NKC + k: b * NKC + k + 1],
                                scale=float(1.0 / np.sqrt(HS)),
                            )
                            if k * 128 >= qlo:
                                nc.vector.tensor_mul(
                                    pt[:, c0:c0 + 128], pt[:, c0:c0 + 128], tri01
                                )
                            for (a, e) in pieces:
                                last = (k == (kpc1 - 1 if e == SEG else kmax - 1))
                                nc.tensor.matmul(
                                    y[:, a:e],
                                    mmcast(Vp[(b, h)][:, k, :]),
                                    mmcast(pt[:, a:e]),
                                    start=(k == 0), stop=last,
                                )
                        d = dn_pool.tile([1, QTW], F32, tag="d")
                        nc.scalar.activation(
                            d, y[HS:HS + 1, :],
                            func=mybir.ActivationFunctionType.Copy,
                        )
                        rd = dn_pool.tile([1, QTW], F32, tag="rd")
                        nc.vector.reciprocal(rd, d)
                        rdb = dn_pool.tile([HS, QTW], F32, tag="rdb")
                        nc.gpsimd.partition_broadcast(rdb, rd, channels=HS)
                        nc.vector.tensor_mul(
                            yTs[(b, h)][:, qlo:qlo + QTW], y[0:HS, :], rdb
                        )
                # output projection for batch b (split-K over the 2 heads)
                for tt in range(T // 128):
                    op = acc.tile([128, C], F32, tag="acc")
                    tsl = slice(tt * 128, (tt + 1) * 128)
                    for a in range(0, C, SEG):
                        nc.tensor.matmul(
                            op[:, a:a + SEG], mmcast(yTs[(b, 0)][:, tsl]),
                            mmcast(wp0[:, a:a + SEG]), start=True, stop=False,
                        )
                        nc.tensor.matmul(
                            op[:, a:a + SEG], mmcast(yTs[(b, 1)][:, tsl]),
                            mmcast(wp1[:, a:a + SEG]), start=False, stop=True,
                        )
                    ob = ob_pool.tile([128, C], F32, tag="ob")
                    nc.vector.tensor_copy(ob, op)
                    nc.sync.dma_start(
                        out=out[b * T + tt * 128: b * T + (tt + 1) * 128, :], in_=ob
                    )

        for p in (ob_pool, dn_pool, pt_pool, xt_pool, persist):
            p.release()

    nc.compile()
    return nc


def _shard_inputs(x, attention_mask, W_attn, b_attn, W_proj):
    xT = np.ascontiguousarray(x.reshape(NTOK, C).T)
    am = np.asarray(attention_mask) != 0
    mb = np.where(am, np.float32(0.0), np.float32(MASK_NEG)).astype(np.float32)
    # [B, T] -> [128, B*NKC] with column index b*NKC + k
    mb = np.ascontiguousarray(mb.reshape(B, NKC, 128).transpose(2, 0, 1).reshape(128, B * NKC))
    in_maps = []
    for c in range(NCORES):
        fs = slice(F * c, F * (c + 1))
        in_maps.append({
            "xT": xT,
            "wq": np.ascontiguousarray(W_attn[:, fs]),
            "wk": np.ascontiguousarray(W_attn[:, C + F * c: C + F * (c + 1)]),
            "wv": np.ascontiguousarray(W_attn[:, 2 * C + F * c: 2 * C + F * (c + 1)]),
            "bq": np.ascontiguousarray(b_attn[fs]).reshape(F, 1),
            "bk": np.ascontiguousarray(b_attn[C + F * c: C + F * (c + 1)]).reshape(F, 1),
            "bv": np.ascontiguousarray(b_attn[2 * C + F * c: 2 * C + F * (c + 1)]).reshape(F, 1),
            "wp": np.ascontiguousarray(W_proj[fs, :]),
            "mbias": mb,
        })
    return in_maps


def get_program(dt_mm_name=None):
    name = dt_mm_name or DT_MM_NAME
    if name not in _prog_cache:
        _prog_cache[name] = _build(name)
    return _prog_cache[name]


def kernel(x, attention_mask, W_attn, b_attn, W_proj, b_proj, **run_kwargs):
    x = np.asarray(x, np.float32)
    W_attn = np.asarray(W_attn, np.float32)
    b_attn = np.asarray(b_attn, np.float32)
    W_proj = np.asarray(W_proj, np.float32)
    b_proj = np.asarray(b_proj, np.float32)

    nc = get_program()
    in_maps = _shard_inputs(x, attention_mask, W_attn, b_attn, W_proj)
    res = run_bass_kernel_spmd(nc, in_maps, core_ids=list(range(NCORES)), **run_kwargs)
    partials = [np.asarray(res.results[i]["out"]) for i in range(NCORES)]
    full = np.sum(np.stack(partials, 0), axis=0, dtype=np.float64).astype(np.float32)
    full = full + b_proj[None, :]
    if run_kwargs:
        kernel.last_result = res
    return full.reshape(B, T, C)


# revision 7
# speedup vs baseline: 1.9440x; 1.9440x over previous
"""Causal self-attention (B=2, T=2048, C=1024, 16 heads x 64) on 8 TRN2 NeuronCores.

Tensor-parallel over heads: each core owns 2 heads. Per core:
  - QKV projection computed in transposed layout (features on partitions)
    from a host-pretransposed x^T, so the contraction dim C sits on
    partitions for every matmul.
  - Scores are computed as S^T (keys on partitions, queries on free dim),
    softmax denominator comes for free from a ones-column appended to V,
    masking folds into the exp activation's per-partition bias, and the
    causal triangle is a single 128x128 multiply per diagonal block.
  - Output projection is split-K over the two heads; partial outputs are
    summed on the host (the all-reduce of row-parallel tensor parallelism).
"""

import os
import sys
import types

import numpy as np

for _p in ("/opt/trn_rl_repo",):
    if os.path.isdir(_p) and _p not in sys.path:
        sys.path.append(_p)

import concourse.bass as bass  # noqa: E402
import concourse.tile as tile  # noqa: E402
from concourse import bacc, mybir  # noqa: E402
from concourse.bass_utils import run_bass_kernel_spmd  # noqa: E402
from concourse.masks import make_identity, make_upper_triangular  # noqa: E402

B, T, C = 2, 2048, 1024
NH, HS = 16, 64
NCORES = 8
HPC = NH // NCORES          # heads per core
F = HPC * HS                # per-core qkv feature width (128)
NTOK = B * T                # 4096 flattened tokens
SEG = 512                   # token segment width for the qkv projection
NSEG = NTOK // SEG
NKC = T // 128              # key chunks per batch
QTW = 1024                  # query tile width in the attention loop
NQT = T // QTW
MASK_NEG = -30000.0

F32 = mybir.dt.float32

# "float32r" runs the PE at 4x the float32 rate (moving dim >= 256) with a
# reduced-precision multiply; "float32" is exact. Selected via env for A/B.
DT_MM_NAME = os.environ.get("KERNEL_DT_MM", "float32r")

_prog_cache = {}


def _build(dt_mm_name):
    dt_mm = getattr(mybir.dt, dt_mm_name)

    nc = bacc.Bacc("TRN2", target_bir_lowering=False, debug=False)

    DTM = dt_mm
    xT = nc.declare_dram_parameter("xT", [C, NTOK], DTM, isOutput=False)
    wq = nc.declare_dram_parameter("wq", [C, F], DTM, isOutput=False)
    wk = nc.declare_dram_parameter("wk", [C, F], DTM, isOutput=False)
    wv = nc.declare_dram_parameter("wv", [C, F], DTM, isOutput=False)
    bq = nc.declare_dram_parameter("bq", [F, 1], F32, isOutput=False)
    bk = nc.declare_dram_parameter("bk", [F, 1], F32, isOutput=False)
    bv = nc.declare_dram_parameter("bv", [F, 1], F32, isOutput=False)
    wp = nc.declare_dram_parameter("wp", [F, C], DTM, isOutput=False)
    mbias = nc.declare_dram_parameter("mbias", [128, B * NKC], F32, isOutput=False)
    out = nc.declare_dram_parameter("out", [NTOK, C], F32, isOutput=True)

    with tile.TileContext(nc) as tc:
        persist = tc.alloc_tile_pool(name="persist", bufs=1)
        xt_pool = tc.alloc_tile_pool(name="xt", bufs=4)
        pt_pool = tc.alloc_tile_pool(name="pt", bufs=3)
        dn_pool = tc.alloc_tile_pool(name="dn", bufs=2)
        ob_pool = tc.alloc_tile_pool(name="ob", bufs=3)

        # --- constants / weights ---
        wqT = persist.tile([128, C // 128, F], DTM, tag="wqT")
        wkT = persist.tile([128, C // 128, F], DTM, tag="wkT")
        wvT = persist.tile([128, C // 128, F], DTM, tag="wvT")
        nc.sync.dma_start(out=wqT, in_=wq[:].rearrange("(k p) f -> p k f", p=128))
        nc.sync.dma_start(out=wkT, in_=wk[:].rearrange("(k p) f -> p k f", p=128))
        nc.sync.dma_start(out=wvT, in_=wv[:].rearrange("(k p) f -> p k f", p=128))
        bqc = persist.tile([F, 1], F32, tag="bqc")
        bkc = persist.tile([F, 1], F32, tag="bkc")
        bvc = persist.tile([F, 1], F32, tag="bvc")
        nc.sync.dma_start(out=bqc, in_=bq[:])
        nc.sync.dma_start(out=bkc, in_=bk[:])
        nc.sync.dma_start(out=bvc, in_=bv[:])
        wp0 = persist.tile([HS, C], DTM, tag="wp0")
        wp1 = persist.tile([HS, C], DTM, tag="wp1")
        nc.sync.dma_start(out=wp0, in_=wp[0:HS, :])
        nc.sync.dma_start(out=wp1, in_=wp[HS:F, :])
        mb = persist.tile([128, B * NKC], F32, tag="mb")
        nc.sync.dma_start(out=mb, in_=mbias[:])

        ident = persist.tile([128, 128], F32, tag="ident")
        make_identity(nc, ident)
        tri01 = persist.tile([128, 128], F32, tag="tri01")
        make_upper_triangular(nc, tri01, val=1.0, diag=True)

        QT = persist.tile([F, NTOK], DTM, tag="QT")
        KT = persist.tile([F, NTOK], DTM, tag="KT")
        VT = persist.tile([F, NTOK], F32, tag="VT")

        # --- phase A: q/k/v projections in transposed layout ---
        with tc.tile_pool(name="psA", bufs=2, space="PSUM") as psA:
            for s in range(NSEG):
                sl = slice(s * SEG, (s + 1) * SEG)
                xts = []
                for k in range(C // 128):
                    xt = xt_pool.tile([128, SEG], DTM, tag="xt")
                    nc.sync.dma_start(out=xt, in_=xT[k * 128:(k + 1) * 128, sl])
                    xts.append(xt)
                psq = psA.tile([F, SEG], F32, tag="psq")
                psk = psA.tile([F, SEG], F32, tag="psk")
                psv = psA.tile([F, SEG], F32, tag="psv")
                for k in range(C // 128):
                    f = dict(start=(k == 0), stop=(k == C // 128 - 1))
                    nc.tensor.matmul(psq, wqT[:, k, :], xts[k][:], **f)
                    nc.tensor.matmul(psk, wkT[:, k, :], xts[k][:], **f)
                    nc.tensor.matmul(psv, wvT[:, k, :], xts[k][:], **f)
                nc.vector.tensor_scalar_add(QT[:, sl], psq, bqc)
                nc.vector.tensor_scalar_add(KT[:, sl], psk, bkc)
                nc.vector.tensor_scalar_add(VT[:, sl], psv, bvc)

        # --- phase B: V^T -> V' (keys on partitions, ones column for denom) ---
        Vp = {}
        for b in range(B):
            for h in range(HPC):
                Vp[(b, h)] = persist.tile([128, NKC, HS + 1], DTM, tag=f"vp{b}{h}", name=f"vp{b}{h}")
                nc.gpsimd.memset(Vp[(b, h)][:, :, HS:HS + 1].bitcast(F32), 1.0)
        with tc.tile_pool(name="psB", bufs=2, space="PSUM") as psB:
            for b in range(B):
                for h in range(HPC):
                    hp = slice(h * HS, (h + 1) * HS)
                    for k in range(NKC):
                        src = VT[hp, b * T + k * 128: b * T + (k + 1) * 128]
                        tr = psB.tile([128, HS], F32, tag="tr")
                        nc.tensor.transpose(tr, src, ident[hp, hp])
                        nc.vector.tensor_copy(Vp[(b, h)][:, k, 0:HS], tr)

        # --- phases C+D: attention + output projection ---
        yTs = {}
        for b in range(B):
            for h in range(HPC):
                yTs[(b, h)] = persist.tile([HS, T], DTM, tag=f"yt{b}{h}", name=f"yt{b}{h}")

        with (
            tc.tile_pool(name="psS", bufs=2, space="PSUM") as psS,
            tc.tile_pool(name="acc", bufs=2, space="PSUM") as acc,
        ):
            for b in range(B):
                for h in range(HPC):
                    hp = slice(h * HS, (h + 1) * HS)
                    for qt in range(NQT):
                        qlo = qt * QTW
                        kmax = (qlo + QTW) // 128
                        kpc1 = min(kmax, qlo // 128 + 4)  # last k touching piece 1
                        y = acc.tile([HS + 1, QTW], F32, tag="acc")
                        for k in range(kmax):
                            c0 = max(0, k * 128 - qlo)
                            pieces = (
                                [(c0, SEG), (SEG, QTW)] if c0 < SEG else [(c0, QTW)]
                            )
                            st = psS.tile([128, QTW], F32, tag="st")
                            for (a, e) in pieces:
                                nc.tensor.matmul(
                                    st[:, a:e],
                                    KT[hp, b * T + k * 128: b * T + (k + 1) * 128],
                                    QT[hp, b * T + qlo + a: b * T + qlo + e],
                                    start=True, stop=True,
                                )
                            pt = pt_pool.tile([128, QTW], DTM, tag="pt")
                            nc.scalar.activation(
                                pt[:, c0:QTW], st[:, c0:QTW],
                                func=mybir.ActivationFunctionType.Exp,
                                bias=mb[:, b * NKC + k: b * NKC + k + 1],
                                scale=float(1.0 / np.sqrt(HS)),
                            )
                            if k * 128 >= qlo:
                                nc.vector.tensor_mul(
                                    pt[:, c0:c0 + 128], pt[:, c0:c0 + 128], tri01
                                )
                            for (a, e) in pieces:
                                last = (k == (kpc1 - 1 if e == SEG else kmax - 1))
                                nc.tensor.matmul(
                                    y[:, a:e],
                                    Vp[(b, h)][:, k, :],
                                    pt[:, a:e],
                                    start=(k == 0), stop=last,
                                )
                        d = dn_pool.tile([1, QTW], F32, tag="d")
                        nc.scalar.activation(
                            d, y[HS:HS + 1, :],
                            func=mybir.ActivationFunctionType.Copy,
                        )
                        rd = dn_pool.tile([1, QTW], F32, tag="rd")
                        nc.vector.reciprocal(rd, d)
                        rdb = dn_pool.tile([HS, QTW], F32, tag="rdb")
                        nc.gpsimd.partition_broadcast(rdb, rd, channels=HS)
                        nc.vector.tensor_mul(
                            yTs[(b, h)][:, qlo:qlo + QTW], y[0:HS, :], rdb
                        )
                # output projection for batch b (split-K over the 2 heads)
                for tt in range(T // 128):
                    op = acc.tile([128, C], F32, tag="acc")
                    tsl = slice(tt * 128, (tt + 1) * 128)
                    for a in range(0, C, SEG):
                        nc.tensor.matmul(
                            op[:, a:a + SEG], yTs[(b, 0)][:, tsl],
                            wp0[:, a:a + SEG], start=True, stop=False,
                        )
                        nc.tensor.matmul(
                            op[:, a:a + SEG], yTs[(b, 1)][:, tsl],
                            wp1[:, a:a + SEG], start=False, stop=True,
                        )
                    ob = ob_pool.tile([128, C], F32, tag="ob")
                    nc.vector.tensor_copy(ob, op)
                    nc.sync.dma_start(
                        out=out[b * T + tt * 128: b * T + (tt + 1) * 128, :], in_=ob
                    )

        for p in (ob_pool, dn_pool, pt_pool, xt_pool, persist):
            p.release()

    nc.compile()
    return nc


def _shard_inputs(x, attention_mask, W_attn, b_attn, W_proj):
    xT = np.ascontiguousarray(x.reshape(NTOK, C).T)
    am = np.asarray(attention_mask) != 0
    mb = np.where(am, np.float32(0.0), np.float32(MASK_NEG)).astype(np.float32)
    # [B, T] -> [128, B*NKC] with column index b*NKC + k
    mb = np.ascontiguousarray(mb.reshape(B, NKC, 128).transpose(2, 0, 1).reshape(128, B * NKC))
    in_maps = []
    for c in range(NCORES):
        fs = slice(F * c, F * (c + 1))
        in_maps.append({
            "xT": xT,
            "wq": np.ascontiguousarray(W_attn[:, fs]),
            "wk": np.ascontiguousarray(W_attn[:, C + F * c: C + F * (c + 1)]),
            "wv": np.ascontiguousarray(W_attn[:, 2 * C + F * c: 2 * C + F * (c + 1)]),
            "bq": np.ascontiguousarray(b_attn[fs]).reshape(F, 1),
            "bk": np.ascontiguousarray(b_attn[C + F * c: C + F * (c + 1)]).reshape(F, 1),
            "bv": np.ascontiguousarray(b_attn[2 * C + F * c: 2 * C + F * (c + 1)]).reshape(F, 1),
            "wp": np.ascontiguousarray(W_proj[fs, :]),
            "mbias": mb,
        })
    return in_maps


def get_program(dt_mm_name=None):
    name = dt_mm_name or DT_MM_NAME
    if name not in _prog_cache:
        _prog_cache[name] = _build(name)
    return _prog_cache[name]


def kernel(x, attention_mask, W_attn, b_attn, W_proj, b_proj, **run_kwargs):
    x = np.asarray(x, np.float32)
    W_attn = np.asarray(W_attn, np.float32)
    b_attn = np.asarray(b_attn, np.float32)
    W_proj = np.asarray(W_proj, np.float32)
    b_proj = np.asarray(b_proj, np.float32)

    nc = get_program()
    in_maps = _shard_inputs(x, attention_mask, W_attn, b_attn, W_proj)
    res = run_bass_kernel_spmd(nc, in_maps, core_ids=list(range(NCORES)), **run_kwargs)
    partials = [np.asarray(res.results[i]["out"]) for i in range(NCORES)]
    full = np.sum(np.stack(partials, 0), axis=0, dtype=np.float64).astype(np.float32)
    full = full + b_proj[None, :]
    if run_kwargs:
        kernel.last_result = res
    return full.reshape(B, T, C)


# revision 8
# speedup vs baseline: 2.1771x; 1.1199x over previous
"""Causal self-attention (B=2, T=2048, C=1024, 16 heads x 64) on 8 TRN2 NeuronCores.

Tensor-parallel over heads: each core owns 2 heads. Per core:
  - QKV projection computed in transposed layout (features on partitions)
    from a host-pretransposed x^T, so the contraction dim C sits on
    partitions for every matmul.
  - Scores are computed as S^T (keys on partitions, queries on free dim),
    softmax denominator comes for free from a ones-column appended to V,
    masking folds into the exp activation's per-partition bias, and the
    causal triangle is a single 128x128 multiply per diagonal block.
  - Output projection is split-K over the two heads; partial outputs are
    summed on the host (the all-reduce of row-parallel tensor parallelism).
"""

import os
import sys
import types

import numpy as np

for _p in ("/opt/trn_rl_repo",):
    if os.path.isdir(_p) and _p not in sys.path:
        sys.path.append(_p)

import concourse.bass as bass  # noqa: E402
import concourse.tile as tile  # noqa: E402
from concourse import bacc, mybir  # noqa: E402
from concourse.bass_utils import run_bass_kernel_spmd  # noqa: E402
from concourse.masks import make_identity, make_upper_triangular  # noqa: E402

B, T, C = 2, 2048, 1024
NH, HS = 16, 64
NCORES = 8
HPC = NH // NCORES          # heads per core
F = HPC * HS                # per-core qkv feature width (128)
NTOK = B * T                # 4096 flattened tokens
SEG = 512                   # token segment width for the qkv projection
NSEG = NTOK // SEG
NKC = T // 128              # key chunks per batch
QTW = 1024                  # query tile width in the attention loop
NQT = T // QTW
MASK_NEG = -30000.0

F32 = mybir.dt.float32

# "float32r" runs the PE at 4x the float32 rate (moving dim >= 256) with a
# reduced-precision multiply; "float32" is exact. Selected via env for A/B.
DT_MM_NAME = os.environ.get("KERNEL_DT_MM", "float32r")

_prog_cache = {}


def _build(dt_mm_name):
    dt_mm = getattr(mybir.dt, dt_mm_name)
    # f32r cannot go through gpsimd affine_select/memset (ISA check), so the
    # V-transpose path (VT/psum/identity) stays plain f32 for f32/f32r modes.
    dt_tr = F32 if dt_mm in (F32, mybir.dt.float32r) else dt_mm

    nc = bacc.Bacc("TRN2", target_bir_lowering=False, debug=False)

    DTM = dt_mm
    xT = nc.declare_dram_parameter("xT", [C, NTOK], DTM, isOutput=False)
    wq = nc.declare_dram_parameter("wq", [C, F], DTM, isOutput=False)
    wk = nc.declare_dram_parameter("wk", [C, F], DTM, isOutput=False)
    wv = nc.declare_dram_parameter("wv", [C, F], DTM, isOutput=False)
    bq = nc.declare_dram_parameter("bq", [F, 1], F32, isOutput=False)
    bk = nc.declare_dram_parameter("bk", [F, 1], F32, isOutput=False)
    bv = nc.declare_dram_parameter("bv", [F, 1], F32, isOutput=False)
    wp = nc.declare_dram_parameter("wp", [F, C], DTM, isOutput=False)
    mbias = nc.declare_dram_parameter("mbias", [128, B * NKC], F32, isOutput=False)
    out = nc.declare_dram_parameter("out", [NTOK, C], F32, isOutput=True)

    with tile.TileContext(nc) as tc:
        persist = tc.alloc_tile_pool(name="persist", bufs=1)
        xt_pool = tc.alloc_tile_pool(name="xt", bufs=4)
        pt_pool = tc.alloc_tile_pool(name="pt", bufs=3)
        dn_pool = tc.alloc_tile_pool(name="dn", bufs=2)
        ob_pool = tc.alloc_tile_pool(name="ob", bufs=3)

        # --- constants / weights ---
        wqT = persist.tile([128, C // 128, F], DTM, tag="wqT")
        wkT = persist.tile([128, C // 128, F], DTM, tag="wkT")
        wvT = persist.tile([128, C // 128, F], DTM, tag="wvT")
        nc.sync.dma_start(out=wqT, in_=wq[:].rearrange("(k p) f -> p k f", p=128))
        nc.sync.dma_start(out=wkT, in_=wk[:].rearrange("(k p) f -> p k f", p=128))
        nc.sync.dma_start(out=wvT, in_=wv[:].rearrange("(k p) f -> p k f", p=128))
        bqc = persist.tile([F, 1], F32, tag="bqc")
        bkc = persist.tile([F, 1], F32, tag="bkc")
        bvc = persist.tile([F, 1], F32, tag="bvc")
        nc.sync.dma_start(out=bqc, in_=bq[:])
        nc.sync.dma_start(out=bkc, in_=bk[:])
        nc.sync.dma_start(out=bvc, in_=bv[:])
        wp0 = persist.tile([HS, C], DTM, tag="wp0")
        wp1 = persist.tile([HS, C], DTM, tag="wp1")
        nc.sync.dma_start(out=wp0, in_=wp[0:HS, :])
        nc.sync.dma_start(out=wp1, in_=wp[HS:F, :])
        mb = persist.tile([128, B * NKC], F32, tag="mb")
        nc.sync.dma_start(out=mb, in_=mbias[:])

        ident = persist.tile([128, 128], dt_tr, tag="ident")
        make_identity(nc, ident)
        tri01 = persist.tile([128, 128], dt_tr, tag="tri01")
        make_upper_triangular(nc, tri01, val=1.0, diag=True)

        QT = persist.tile([F, NTOK], DTM, tag="QT")
        KT = persist.tile([F, NTOK], DTM, tag="KT")
        VT = persist.tile([F, NTOK], dt_tr, tag="VT")

        # --- phase A: q/k/v projections in transposed layout ---
        with tc.tile_pool(name="psA", bufs=2, space="PSUM") as psA:
            for s in range(NSEG):
                sl = slice(s * SEG, (s + 1) * SEG)
                xts = []
                for k in range(C // 128):
                    xt = xt_pool.tile([128, SEG], DTM, tag="xt")
                    nc.sync.dma_start(out=xt, in_=xT[k * 128:(k + 1) * 128, sl])
                    xts.append(xt)
                psq = psA.tile([F, SEG], F32, tag="psq")
                psk = psA.tile([F, SEG], F32, tag="psk")
                psv = psA.tile([F, SEG], F32, tag="psv")
                for k in range(C // 128):
                    f = dict(start=(k == 0), stop=(k == C // 128 - 1))
                    nc.tensor.matmul(psq, wqT[:, k, :], xts[k][:], **f)
                    nc.tensor.matmul(psk, wkT[:, k, :], xts[k][:], **f)
                    nc.tensor.matmul(psv, wvT[:, k, :], xts[k][:], **f)
                nc.vector.tensor_scalar_add(QT[:, sl], psq, bqc)
                nc.vector.tensor_scalar_add(KT[:, sl], psk, bkc)
                nc.vector.tensor_scalar_add(VT[:, sl], psv, bvc)

        # --- phase B: V^T -> V' (keys on partitions, ones column for denom) ---
        Vp = {}
        for b in range(B):
            for h in range(HPC):
                Vp[(b, h)] = persist.tile([128, NKC, HS + 1], DTM, tag=f"vp{b}{h}", name=f"vp{b}{h}")
                ones_col = Vp[(b, h)][:, :, HS:HS + 1]
                if dt_mm is mybir.dt.float32r:
                    ones_col = ones_col.bitcast(F32)
                nc.gpsimd.memset(ones_col, 1.0)
        with tc.tile_pool(name="psB", bufs=2, space="PSUM") as psB:
            for b in range(B):
                for h in range(HPC):
                    hp = slice(h * HS, (h + 1) * HS)
                    for k in range(NKC):
                        src = VT[hp, b * T + k * 128: b * T + (k + 1) * 128]
                        tr = psB.tile([128, HS], dt_tr, tag="tr")
                        nc.tensor.transpose(tr, src, ident[hp, hp])
                        nc.vector.tensor_copy(Vp[(b, h)][:, k, 0:HS], tr)

        # --- phases C+D: attention + output projection ---
        yTs = {}
        for b in range(B):
            for h in range(HPC):
                yTs[(b, h)] = persist.tile([HS, T], DTM, tag=f"yt{b}{h}", name=f"yt{b}{h}")

        with (
            tc.tile_pool(name="psS", bufs=2, space="PSUM") as psS,
            tc.tile_pool(name="acc", bufs=2, space="PSUM") as acc,
        ):
            for b in range(B):
                for h in range(HPC):
                    hp = slice(h * HS, (h + 1) * HS)
                    for qt in range(NQT):
                        qlo = qt * QTW
                        kmax = (qlo + QTW) // 128
                        kpc1 = min(kmax, qlo // 128 + 4)  # last k touching piece 1
                        y = acc.tile([HS + 1, QTW], F32, tag="acc")
                        for k in range(kmax):
                            c0 = max(0, k * 128 - qlo)
                            pieces = (
                                [(c0, SEG), (SEG, QTW)] if c0 < SEG else [(c0, QTW)]
                            )
                            st = psS.tile([128, QTW], F32, tag="st")
                            for (a, e) in pieces:
                                nc.tensor.matmul(
                                    st[:, a:e],
                                    KT[hp, b * T + k * 128: b * T + (k + 1) * 128],
                                    QT[hp, b * T + qlo + a: b * T + qlo + e],
                                    start=True, stop=True,
                                )
                            pt = pt_pool.tile([128, QTW], DTM, tag="pt")
                            nc.scalar.activation(
                                pt[:, c0:QTW], st[:, c0:QTW],
                                func=mybir.ActivationFunctionType.Exp,
                                bias=mb[:, b * NKC + k: b * NKC + k + 1],
                                scale=float(1.0 / np.sqrt(HS)),
                            )
                            if k * 128 >= qlo:
                                nc.vector.tensor_mul(
                                    pt[:, c0:c0 + 128], pt[:, c0:c0 + 128], tri01
                                )
                            for (a, e) in pieces:
                                last = (k == (kpc1 - 1 if e == SEG else kmax - 1))
                                nc.tensor.matmul(
                                    y[:, a:e],
                                    Vp[(b, h)][:, k, :],
                                    pt[:, a:e],
                                    start=(k == 0), stop=last,
                                )
                        d = dn_pool.tile([1, QTW], F32, tag="d")
                        nc.scalar.activation(
                            d, y[HS:HS + 1, :],
                            func=mybir.ActivationFunctionType.Copy,
                        )
                        rd = dn_pool.tile([1, QTW], F32, tag="rd")
                        nc.vector.reciprocal(rd, d)
                        rdb = dn_pool.tile([HS, QTW], F32, tag="rdb")
                        nc.gpsimd.partition_broadcast(rdb, rd, channels=HS)
                        nc.vector.tensor_mul(
                            yTs[(b, h)][:, qlo:qlo + QTW], y[0:HS, :], rdb
                        )
                # output projection for batch b (split-K over the 2 heads)
                for tt in range(T // 128):
                    op = acc.tile([128, C], F32, tag="acc")
                    tsl = slice(tt * 128, (tt + 1) * 128)
                    for a in range(0, C, SEG):
                        nc.tensor.matmul(
                            op[:, a:a + SEG], yTs[(b, 0)][:, tsl],
                            wp0[:, a:a + SEG], start=True, stop=False,
                        )
                        nc.tensor.matmul(
                            op[:, a:a + SEG], yTs[(b, 1)][:, tsl],
                            wp1[:, a:a + SEG], start=False, stop=True,
                        )
                    ob = ob_pool.tile([128, C], F32, tag="ob")
                    nc.vector.tensor_copy(ob, op)
                    nc.sync.dma_start(
                        out=out[b * T + tt * 128: b * T + (tt + 1) * 128, :], in_=ob
                    )

        for p in (ob_pool, dn_pool, pt_pool, xt_pool, persist):
            p.release()

    nc.compile()
    return nc


def _np_mm_dtype(name):
    if name in ("float32", "float32r"):
        return np.float32
    if name == "float16":
        return np.float16
    import ml_dtypes

    return np.dtype(getattr(ml_dtypes, name))


def _shard_inputs(x, attention_mask, W_attn, b_attn, W_proj, npdt=np.float32):
    xT = np.ascontiguousarray(x.reshape(NTOK, C).T.astype(npdt))
    am = np.asarray(attention_mask) != 0
    mb = np.where(am, np.float32(0.0), np.float32(MASK_NEG)).astype(np.float32)
    # [B, T] -> [128, B*NKC] with column index b*NKC + k
    mb = np.ascontiguousarray(mb.reshape(B, NKC, 128).transpose(2, 0, 1).reshape(128, B * NKC))
    in_maps = []
    for c in range(NCORES):
        fs = slice(F * c, F * (c + 1))
        in_maps.append({
            "xT": xT,
            "wq": np.ascontiguousarray(W_attn[:, fs].astype(npdt)),
            "wk": np.ascontiguousarray(W_attn[:, C + F * c: C + F * (c + 1)].astype(npdt)),
            "wv": np.ascontiguousarray(W_attn[:, 2 * C + F * c: 2 * C + F * (c + 1)].astype(npdt)),
            "bq": np.ascontiguousarray(b_attn[fs]).reshape(F, 1),
            "bk": np.ascontiguousarray(b_attn[C + F * c: C + F * (c + 1)]).reshape(F, 1),
            "bv": np.ascontiguousarray(b_attn[2 * C + F * c: 2 * C + F * (c + 1)]).reshape(F, 1),
            "wp": np.ascontiguousarray(W_proj[fs, :].astype(npdt)),
            "mbias": mb,
        })
    return in_maps


def get_program(dt_mm_name=None):
    name = dt_mm_name or DT_MM_NAME
    if name not in _prog_cache:
        _prog_cache[name] = _build(name)
    return _prog_cache[name]


def kernel(x, attention_mask, W_attn, b_attn, W_proj, b_proj, **run_kwargs):
    x = np.asarray(x, np.float32)
    W_attn = np.asarray(W_attn, np.float32)
    b_attn = np.asarray(b_attn, np.float32)
    W_proj = np.asarray(W_proj, np.float32)
    b_proj = np.asarray(b_proj, np.float32)

    nc = get_program()
    in_maps = _shard_inputs(
        x, attention_mask, W_attn, b_attn, W_proj, npdt=_np_mm_dtype(DT_MM_NAME)
    )
    res = run_bass_kernel_spmd(nc, in_maps, core_ids=list(range(NCORES)), **run_kwargs)
    partials = [np.asarray(res.results[i]["out"]) for i in range(NCORES)]
    full = np.sum(np.stack(partials, 0), axis=0, dtype=np.float64).astype(np.float32)
    full = full + b_proj[None, :]
    if run_kwargs:
        kernel.last_result = res
    return full.reshape(B, T, C)


# revision 10
# speedup vs baseline: 2.3531x; 1.0809x over previous
"""Causal self-attention (B=2, T=2048, C=1024, 16 heads x 64) on 8 TRN2 NeuronCores.

Tensor-parallel over heads: each core owns 2 heads. Per core:
  - QKV projection computed in transposed layout (features on partitions)
    from a host-pretransposed x^T, so the contraction dim C sits on
    partitions for every matmul.
  - Scores are computed as S^T (keys on partitions, queries on free dim),
    softmax denominator comes for free from a ones-column appended to V,
    masking folds into the exp activation's per-partition bias, and the
    causal triangle is a single 128x128 multiply per diagonal block.
  - Output projection is split-K over the two heads; partial outputs are
    summed on the host (the all-reduce of row-parallel tensor parallelism).
"""

import os
import sys
import types

import numpy as np

for _p in ("/opt/trn_rl_repo",):
    if os.path.isdir(_p) and _p not in sys.path:
        sys.path.append(_p)

import concourse.bass as bass  # noqa: E402
import concourse.tile as tile  # noqa: E402
from concourse import bacc, mybir  # noqa: E402
from concourse.bass_utils import run_bass_kernel_spmd  # noqa: E402
from concourse.masks import make_identity, make_upper_triangular  # noqa: E402

B, T, C = 2, 2048, 1024
NH, HS = 16, 64
NCORES = 8
HPC = NH // NCORES          # heads per core
F = HPC * HS                # per-core qkv feature width (128)
NTOK = B * T                # 4096 flattened tokens
SEG = 512                   # token segment width for the qkv projection
NSEG = NTOK // SEG
NKC = T // 128              # key chunks per batch
QTW = 1024                  # query tile width in the attention loop
NQT = T // QTW
MASK_NEG = -30000.0

F32 = mybir.dt.float32

# "float32r" runs the PE at 4x the float32 rate (moving dim >= 256) with a
# reduced-precision multiply; "float32" is exact. Selected via env for A/B.
DT_MM_NAME = os.environ.get("KERNEL_DT_MM", "float32r")

_prog_cache = {}


def _build(dt_mm_name):
    dt_mm = getattr(mybir.dt, dt_mm_name)
    # f32r cannot go through gpsimd affine_select/memset (ISA check), so the
    # V-transpose path (VT/psum/identity) stays plain f32 for f32/f32r modes.
    dt_tr = F32 if dt_mm in (F32, mybir.dt.float32r) else dt_mm

    nc = bacc.Bacc("TRN2", target_bir_lowering=False, debug=False)

    DTM = dt_mm
    xT = nc.declare_dram_parameter("xT", [C, NTOK], DTM, isOutput=False)
    wq = nc.declare_dram_parameter("wq", [C, F], DTM, isOutput=False)
    wk = nc.declare_dram_parameter("wk", [C, F], DTM, isOutput=False)
    wv = nc.declare_dram_parameter("wv", [C, F], DTM, isOutput=False)
    bq = nc.declare_dram_parameter("bq", [F, 1], F32, isOutput=False)
    bk = nc.declare_dram_parameter("bk", [F, 1], F32, isOutput=False)
    bv = nc.declare_dram_parameter("bv", [F, 1], F32, isOutput=False)
    wp = nc.declare_dram_parameter("wp", [F, C], DTM, isOutput=False)
    mbias = nc.declare_dram_parameter("mbias", [128, B * NKC], F32, isOutput=False)
    out = nc.declare_dram_parameter("out", [NTOK, C], F32, isOutput=True)

    with tile.TileContext(nc) as tc:
        persist = tc.alloc_tile_pool(name="persist", bufs=1)
        xt_pool = tc.alloc_tile_pool(name="xt", bufs=4)
        pt_pool = tc.alloc_tile_pool(name="pt", bufs=3)
        dn_pool = tc.alloc_tile_pool(name="dn", bufs=2)
        ob_pool = tc.alloc_tile_pool(name="ob", bufs=3)

        # --- constants / weights ---
        wqT = persist.tile([128, C // 128, F], DTM, tag="wqT")
        wkT = persist.tile([128, C // 128, F], DTM, tag="wkT")
        wvT = persist.tile([128, C // 128, F], DTM, tag="wvT")
        nc.sync.dma_start(out=wqT, in_=wq[:].rearrange("(k p) f -> p k f", p=128))
        nc.sync.dma_start(out=wkT, in_=wk[:].rearrange("(k p) f -> p k f", p=128))
        nc.sync.dma_start(out=wvT, in_=wv[:].rearrange("(k p) f -> p k f", p=128))
        bqc = persist.tile([F, 1], F32, tag="bqc")
        bkc = persist.tile([F, 1], F32, tag="bkc")
        bvc = persist.tile([F, 1], F32, tag="bvc")
        nc.sync.dma_start(out=bqc, in_=bq[:])
        nc.sync.dma_start(out=bkc, in_=bk[:])
        nc.sync.dma_start(out=bvc, in_=bv[:])
        wp0 = persist.tile([HS, C], DTM, tag="wp0")
        wp1 = persist.tile([HS, C], DTM, tag="wp1")
        nc.sync.dma_start(out=wp0, in_=wp[0:HS, :])
        nc.sync.dma_start(out=wp1, in_=wp[HS:F, :])
        mb = persist.tile([128, B * NKC], F32, tag="mb")
        nc.sync.dma_start(out=mb, in_=mbias[:])

        ident = persist.tile([128, 128], dt_tr, tag="ident")
        make_identity(nc, ident)
        identF = persist.tile([128, 128], F32, tag="identF")
        make_identity(nc, identF)
        tri01 = persist.tile([128, 128], dt_tr, tag="tri01")
        make_upper_triangular(nc, tri01, val=1.0, diag=True)

        QT = persist.tile([F, NTOK], DTM, tag="QT")
        KT = persist.tile([F, NTOK], DTM, tag="KT")
        VT = persist.tile([F, NTOK], dt_tr, tag="VT")

        # --- phase A: q/k/v projections in transposed layout ---
        with tc.tile_pool(name="psA", bufs=2, space="PSUM") as psA:
            for s in range(NSEG):
                sl = slice(s * SEG, (s + 1) * SEG)
                xts = []
                for k in range(C // 128):
                    xt = xt_pool.tile([128, SEG], DTM, tag="xt")
                    nc.sync.dma_start(out=xt, in_=xT[k * 128:(k + 1) * 128, sl])
                    xts.append(xt)
                psq = psA.tile([F, SEG], F32, tag="psq")
                psk = psA.tile([F, SEG], F32, tag="psk")
                psv = psA.tile([F, SEG], F32, tag="psv")
                for k in range(C // 128):
                    f = dict(start=(k == 0), stop=(k == C // 128 - 1))
                    nc.tensor.matmul(psq, wqT[:, k, :], xts[k][:], **f)
                    nc.tensor.matmul(psk, wkT[:, k, :], xts[k][:], **f)
                    nc.tensor.matmul(psv, wvT[:, k, :], xts[k][:], **f)
                nc.vector.tensor_scalar_add(QT[:, sl], psq, bqc)
                nc.vector.tensor_scalar_add(KT[:, sl], psk, bkc)
                nc.vector.tensor_scalar_add(VT[:, sl], psv, bvc)

        # --- phase B: V^T -> V' (keys on partitions, ones column for denom) ---
        Vp = {}
        for b in range(B):
            for h in range(HPC):
                Vp[(b, h)] = persist.tile([128, NKC, HS + 1], DTM, tag=f"vp{b}{h}", name=f"vp{b}{h}")
                ones_col = Vp[(b, h)][:, :, HS:HS + 1]
                if dt_mm is mybir.dt.float32r:
                    ones_col = ones_col.bitcast(F32)
                nc.gpsimd.memset(ones_col, 1.0)
        with tc.tile_pool(name="psB", bufs=2, space="PSUM") as psB:
            for b in range(B):
                for h in range(HPC):
                    hp = slice(h * HS, (h + 1) * HS)
                    for k in range(NKC):
                        src = VT[hp, b * T + k * 128: b * T + (k + 1) * 128]
                        tr = psB.tile([128, HS], dt_tr, tag="tr")
                        nc.tensor.transpose(tr, src, ident[hp, hp])
                        nc.vector.tensor_copy(Vp[(b, h)][:, k, 0:HS], tr)

        # --- phases C+D: attention + output projection ---
        yTs = {}
        for b in range(B):
            for h in range(HPC):
                yTs[(b, h)] = persist.tile([HS, T], DTM, tag=f"yt{b}{h}", name=f"yt{b}{h}")

        with (
            tc.tile_pool(name="psS", bufs=2, space="PSUM") as psS,
            tc.tile_pool(name="acc", bufs=2, space="PSUM") as acc,
        ):
            for b in range(B):
                for h in range(HPC):
                    hp = slice(h * HS, (h + 1) * HS)
                    for qt in range(NQT):
                        qlo = qt * QTW
                        kmax = (qlo + QTW) // 128
                        kpc1 = min(kmax, qlo // 128 + 4)  # last k touching piece 1
                        y = acc.tile([HS + 1, QTW], F32, tag="acc")
                        for k in range(kmax):
                            c0 = max(0, k * 128 - qlo)
                            pieces = (
                                [(c0, SEG), (SEG, QTW)] if c0 < SEG else [(c0, QTW)]
                            )
                            st = psS.tile([128, QTW], F32, tag="st")
                            for (a, e) in pieces:
                                nc.tensor.matmul(
                                    st[:, a:e],
                                    KT[hp, b * T + k * 128: b * T + (k + 1) * 128],
                                    QT[hp, b * T + qlo + a: b * T + qlo + e],
                                    start=True, stop=True,
                                )
                            pt = pt_pool.tile([128, QTW], DTM, tag="pt")
                            nc.scalar.activation(
                                pt[:, c0:QTW], st[:, c0:QTW],
                                func=mybir.ActivationFunctionType.Exp,
                                bias=mb[:, b * NKC + k: b * NKC + k + 1],
                                scale=float(1.0 / np.sqrt(HS)),
                            )
                            if k * 128 >= qlo:
                                nc.vector.tensor_mul(
                                    pt[:, c0:c0 + 128], pt[:, c0:c0 + 128], tri01
                                )
                            for (a, e) in pieces:
                                last = (k == (kpc1 - 1 if e == SEG else kmax - 1))
                                nc.tensor.matmul(
                                    y[:, a:e],
                                    Vp[(b, h)][:, k, :],
                                    pt[:, a:e],
                                    start=(k == 0), stop=last,
                                )
                        # evict y' to SBUF promptly (frees the PSUM bank),
                        # then invert the denominator in a PE-transposed
                        # [128, 8] layout so DVE gets full lane parallelism.
                        ysb = dn_pool.tile([HS + 1, QTW], F32, tag="ysb")
                        nc.vector.tensor_copy(ysb, y)
                        dT = psS.tile([128, QTW // 128], F32, tag="st", name="dT")
                        for j in range(QTW // 128):
                            nc.tensor.transpose(
                                dT[:, j:j + 1],
                                ysb[HS:HS + 1, j * 128:(j + 1) * 128],
                                identF[HS:HS + 1, HS:HS + 1],
                            )
                        rdT = dn_pool.tile([128, QTW // 128], F32, tag="rdT")
                        nc.vector.reciprocal(rdT, dT)
                        rdTT = psS.tile([QTW // 128, QTW], F32, tag="st", name="rdTT")
                        nc.tensor.transpose(
                            rdTT[:, 0:128], rdT, identF
                        )
                        tmp8 = dn_pool.tile([QTW // 128, 128], F32, tag="tmp8")
                        nc.vector.tensor_copy(tmp8, rdTT[:, 0:128])
                        rd1 = dn_pool.tile([1, QTW], F32, tag="rd1")
                        nc.sync.dma_start(out=rd1, in_=tmp8)
                        rdb = dn_pool.tile([HS, QTW], F32, tag="rdb")
                        nc.gpsimd.partition_broadcast(rdb, rd1, channels=HS)
                        nc.vector.tensor_mul(
                            yTs[(b, h)][:, qlo:qlo + QTW], ysb[0:HS, :], rdb
                        )
                # output projection for batch b (split-K over the 2 heads)
                for tt in range(T // 128):
                    op = acc.tile([128, C], F32, tag="acc")
                    tsl = slice(tt * 128, (tt + 1) * 128)
                    for a in range(0, C, SEG):
                        nc.tensor.matmul(
                            op[:, a:a + SEG], yTs[(b, 0)][:, tsl],
                            wp0[:, a:a + SEG], start=True, stop=False,
                        )
                        nc.tensor.matmul(
                            op[:, a:a + SEG], yTs[(b, 1)][:, tsl],
                            wp1[:, a:a + SEG], start=False, stop=True,
                        )
                    ob = ob_pool.tile([128, C], F32, tag="ob")
                    nc.vector.tensor_copy(ob, op)
                    nc.sync.dma_start(
                        out=out[b * T + tt * 128: b * T + (tt + 1) * 128, :], in_=ob
                    )

        for p in (ob_pool, dn_pool, pt_pool, xt_pool, persist):
            p.release()

    nc.compile()
    return nc


def _np_mm_dtype(name):
    if name in ("float32", "float32r"):
        return np.float32
    if name == "float16":
        return np.float16
    import ml_dtypes

    return np.dtype(getattr(ml_dtypes, name))


def _shard_inputs(x, attention_mask, W_attn, b_attn, W_proj, npdt=np.float32):
    xT = np.ascontiguousarray(x.reshape(NTOK, C).T.astype(npdt))
    am = np.asarray(attention_mask) != 0
    mb = np.where(am, np.float32(0.0), np.float32(MASK_NEG)).astype(np.float32)
    # [B, T] -> [128, B*NKC] with column index b*NKC + k
    mb = np.ascontiguousarray(mb.reshape(B, NKC, 128).transpose(2, 0, 1).reshape(128, B * NKC))
    in_maps = []
    for c in range(NCORES):
        fs = slice(F * c, F * (c + 1))
        in_maps.append({
            "xT": xT,
            "wq": np.ascontiguousarray(W_attn[:, fs].astype(npdt)),
            "wk": np.ascontiguousarray(W_attn[:, C + F * c: C + F * (c + 1)].astype(npdt)),
            "wv": np.ascontiguousarray(W_attn[:, 2 * C + F * c: 2 * C + F * (c + 1)].astype(npdt)),
            "bq": np.ascontiguousarray(b_attn[fs]).reshape(F, 1),
            "bk": np.ascontiguousarray(b_attn[C + F * c: C + F * (c + 1)]).reshape(F, 1),
            "bv": np.ascontiguousarray(b_attn[2 * C + F * c: 2 * C + F * (c + 1)]).reshape(F, 1),
            "wp": np.ascontiguousarray(W_proj[fs, :].astype(npdt)),
            "mbias": mb,
        })
    return in_maps


def get_program(dt_mm_name=None):
    name = dt_mm_name or DT_MM_NAME
    if name not in _prog_cache:
        _prog_cache[name] = _build(name)
    return _prog_cache[name]


def kernel(x, attention_mask, W_attn, b_attn, W_proj, b_proj, **run_kwargs):
    x = np.asarray(x, np.float32)
    W_attn = np.asarray(W_attn, np.float32)
    b_attn = np.asarray(b_attn, np.float32)
    W_proj = np.asarray(W_proj, np.float32)
    b_proj = np.asarray(b_proj, np.float32)

    nc = get_program()
    in_maps = _shard_inputs(
        x, attention_mask, W_attn, b_attn, W_proj, npdt=_np_mm_dtype(DT_MM_NAME)
    )
    res = run_bass_kernel_spmd(nc, in_maps, core_ids=list(range(NCORES)), **run_kwargs)
    partials = [np.asarray(res.results[i]["out"]) for i in range(NCORES)]
    full = np.sum(np.stack(partials, 0), axis=0, dtype=np.float64).astype(np.float32)
    full = full + b_proj[None, :]
    if run_kwargs:
        kernel.last_result = res
    return full.reshape(B, T, C)
